# revision 1
# baseline (speedup 1.0000x reference)
"""Multi-head self-attention (N=4, T=2048, D=1024, H=16) on 8 TRN2 NeuronCores.

Sharding: core c -> (batch n = c//2, head-group g = c%2 of 8 heads).
Each core projects its batch with its 512-row slices of Wq/Wk/Wv, runs
attention for its 8 heads, AllGathers the per-pair context (bf16, in two
halves so transfer overlaps attention), and computes its 512 output
columns with its slice of Wo.

Kernel structure per core:
  - X^T via PE transpose; projections emit feature-major Q^T/K^T (f32r)
    and token-major V (bf16) with a ones column per head so the softmax
    denominator falls out of the ctx matmul for free.
  - Masked query columns of Q^T are zeroed: their scores are all equal,
    so unnormalized softmax gives exactly the uniform attention the
    reference's -1e20 masked_fill produces.
  - S^T = K^T.T @ Q^T per (head, tq-half); exp on ScalarE PSUM->SBUF in
    bf16; ctx^T (+Z row) = [V|1].T @ P^T; normalize with 1/Z broadcast.
  - ctx matmuls run one unit behind S/exp so ScalarE never starves.
"""

from contextlib import ExitStack

import numpy as np

import concourse.bass as bass
import concourse.mybir as mybir
import concourse.tile as tile
from concourse import bacc
from concourse.bass_utils import run_bass_kernel_spmd
from concourse.masks import make_identity

N, T, D, H, DH = 4, 2048, 1024, 16, 64
N_CORES = 8
G = 512            # per-core projection width (8 heads x 64)
HPC = 8            # heads per core
SCALE = 1.0 / 8.0  # 1/sqrt(DH)

f32 = mybir.dt.float32
f32r = mybir.dt.float32r
bf16 = mybir.dt.bfloat16
i32 = mybir.dt.int32

COMPUTE_DT = "f32r"  # {"f32r", "bf16"} dtype for projection/S matmul operands

# global din-block order produced by the two half-AllGathers:
# half 0 carries heads 0-3 (blocks 0,1) + peer heads 8-11 (blocks 4,5)
CC_PERM = [[0, 1, 4, 5], [2, 3, 6, 7]]


def build_nc(compute_dt: str = COMPUTE_DT, single_core: bool = False,
             reps: int = 0) -> bacc.Bacc:
    # float32r tiles: PE runs 1 cycle/row (vs 4 for fp32); producers
    # (DVE/ACT evicts) round to f32r precision on write.
    cdt = f32r if compute_dt == "f32r" else bf16
    # bf16 halves X^T/Q^T/K^T so the V projection can overlap the first
    # attention units' exp (ScalarE head start); f32r doesn't fit. Measured
    # slower in the cost model (pipeline bubbles around the V window), so
    # disabled; kept for reference.
    overlap = False

    nc = bacc.Bacc(
        "TRN2", target_bir_lowering=False, debug=False, num_devices=N_CORES
    )
    x_d = nc.dram_tensor("query", [T, D], f32, kind="ExternalInput").ap()
    m_d = nc.dram_tensor("mask", [T], i32, kind="ExternalInput").ap()
    wq_d = nc.dram_tensor("Wq", [G, D], f32, kind="ExternalInput").ap()
    wk_d = nc.dram_tensor("Wk", [G, D], f32, kind="ExternalInput").ap()
    wv_d = nc.dram_tensor("Wv", [G, D], f32, kind="ExternalInput").ap()
    wo_d = nc.dram_tensor("Wo", [G, D], f32, kind="ExternalInput").ap()
    bq_d = nc.dram_tensor("bq", [G], f32, kind="ExternalInput").ap()
    bk_d = nc.dram_tensor("bk", [G], f32, kind="ExternalInput").ap()
    bv_d = nc.dram_tensor("bv", [G], f32, kind="ExternalInput").ap()
    bo_d = nc.dram_tensor("bo", [G], f32, kind="ExternalInput").ap()
    out_d = nc.dram_tensor("out", [T, G], f32, kind="ExternalOutput").ap()

    TB = T // 128   # 16 token blocks
    DB = D // 128   # 8 feature blocks
    GB = G // 128   # 4 projected blocks

    with tile.TileContext(nc) as tc, ExitStack() as outer_ctx:
        if reps:
            outer_ctx.enter_context(tc.For_i(0, reps, 1))
        ctx = outer_ctx.enter_context(ExitStack())
        const = ctx.enter_context(tc.tile_pool(name="const", bufs=1))
        identity = const.tile([128, 128], f32)
        make_identity(nc, identity)
        bqk = const.tile([128, 2 * GB], f32, tag="bqk")
        bq_c, bk_c = bqk[:, 0:GB], bqk[:, GB:2 * GB]
        bvo = const.tile([128, 2 * G], f32, tag="bvo")
        bvb, bob = bvo[:, 0:G], bvo[:, G:2 * G]

        qpool = ctx.enter_context(tc.tile_pool(name="qpool", bufs=1))
        q_t = [qpool.tile([128, T], cdt, tag=f"q{i}", name=f"q{i}")
               for i in range(GB)]
        k_t = [qpool.tile([128, T], cdt, tag=f"k{i}", name=f"k{i}")
               for i in range(GB)]
        v_all = qpool.tile([128, TB * HPC * 65], bf16, tag="v_all")
        v_t = [v_all[:, i * HPC * 65:(i + 1) * HPC * 65] for i in range(TB)]

        dram = ctx.enter_context(tc.tile_pool(name="dram", bufs=1, space="DRAM"))
        cc_in = dram.tile([G, T], bf16)
        cc_out = [dram.tile([G, T], bf16, name=f"cc_out{c}", tag=f"cc_out{c}")
                  for c in range(2)]

        maskp = ctx.enter_context(ExitStack())
        mkpool = maskp.enter_context(tc.tile_pool(name="maskp", bufs=1))
        maskb = mkpool.tile([128, T], f32, tag="maskb")

        # ---- mask + biases ----
        with tc.tile_pool(name="mload", bufs=1) as mp:
            m_i = mp.tile([1, T], i32)
            nc.sync.dma_start(m_i[:], m_d[None, :])
            m_f = mp.tile([1, T], f32)
            nc.vector.tensor_copy(m_f[:], m_i[:])
            nc.gpsimd.partition_broadcast(maskb[:], m_f[:])
            nc.sync.dma_start(bq_c[:], bq_d.rearrange("(j p) -> p j", p=128))
            nc.sync.dma_start(bk_c[:], bk_d.rearrange("(j p) -> p j", p=128))
            bv_r = mp.tile([1, G], f32, tag="bvr")
            nc.sync.dma_start(bv_r[:], bv_d[None, :])
            nc.gpsimd.partition_broadcast(bvb[:], bv_r[:])
            bo_r = mp.tile([1, G], f32, tag="bor")
            nc.sync.dma_start(bo_r[:], bo_d[None, :])
            nc.gpsimd.partition_broadcast(bob[:], bo_r[:])

        # ---- phase 1: X^T, W^T, projections ----
        # pool stack must release LIFO: in overlap mode the attention pools
        # are created FIRST so phase-1 pools can release mid-attention.
        att = ctx.enter_context(ExitStack())

        def open_att_pools():
            slabp = att.enter_context(tc.tile_pool(name="slab", bufs=2))
            zp = att.enter_context(tc.tile_pool(name="zbuf", bufs=2))
            csp = att.enter_context(
                tc.tile_pool(name="cstage", bufs=2 if overlap else 3))
            spp = att.enter_context(
                tc.tile_pool(name="spsum", bufs=2, space="PSUM"))
            return slabp, zp, csp, spp

        if overlap:
            att_pools = open_att_pools()
        p1 = ctx.enter_context(ExitStack())
        if True:
            xtp = p1.enter_context(tc.tile_pool(name="xt", bufs=1))
            sp = p1.enter_context(
                tc.tile_pool(name="stage", bufs=2 if overlap else 3))
            wtp = p1.enter_context(
                tc.tile_pool(name="wt", bufs=8 if overlap else 16))
            pp = p1.enter_context(
                tc.tile_pool(name="pp", bufs=2 if overlap else 4,
                             space="PSUM"))
            xt = [xtp.tile([128, T], cdt, tag=f"xt{d}", name=f"xt{d}")
                  for d in range(DB)]
            for i in range(TB):
                xs = sp.tile([128, D], f32, tag="stage", name="stage")
                if i < 2:
                    # split the pipe-filling loads so the first transpose
                    # can start as early as possible
                    for ii in range(4):
                        nc.sync.dma_start(
                            xs[ii * 32:(ii + 1) * 32, :],
                            x_d[i * 128 + ii * 32:i * 128 + (ii + 1) * 32, :],
                        )
                else:
                    nc.sync.dma_start(xs[:], x_d[i * 128:(i + 1) * 128, :])
                for d in range(DB):
                    ps = pp.tile([128, 512], f32, tag="pp", name="pp")
                    nc.tensor.transpose(
                        ps[:, 0:128], xs[:, d * 128:(d + 1) * 128], identity[:]
                    )
                    nc.any.tensor_copy(
                        xt[d][:, i * 128:(i + 1) * 128], ps[:, 0:128]
                    )

            def load_wT(w_dram):
                tiles = [wtp.tile([128, G], cdt, tag="wt", name="wt")
                         for _ in range(DB)]
                for r in range(GB):
                    ws = sp.tile([128, D], f32, tag="stage", name="stage")
                    nc.sync.dma_start(ws[:], w_dram[r * 128:(r + 1) * 128, :])
                    for d in range(DB):
                        ps = pp.tile([128, 512], f32, tag="pp", name="pp")
                        nc.tensor.transpose(
                            ps[:, 0:128], ws[:, d * 128:(d + 1) * 128],
                            identity[:]
                        )
                        nc.any.tensor_copy(
                            tiles[d][:, r * 128:(r + 1) * 128], ps[:, 0:128]
                        )
                return tiles

            # Q^T with masked-query columns zeroed (-> uniform softmax rows,
            # matching the reference's -1e20 masked_fill exactly)
            wqT = load_wT(wq_d)
            for b in range(GB):
                for tch in range(4):
                    ps = pp.tile([128, 512], f32, tag="pp", name="pp")
                    for d in range(DB):
                        nc.tensor.matmul(
                            ps[:],
                            wqT[d][:, b * 128:(b + 1) * 128],
                            xt[d][:, tch * 512:(tch + 1) * 512],
                            start=(d == 0),
                            stop=(d == DB - 1),
                        )
                    nc.vector.scalar_tensor_tensor(
                        q_t[b][:, tch * 512:(tch + 1) * 512],
                        ps[:],
                        bq_c[:, b:b + 1],
                        maskb[:, tch * 512:(tch + 1) * 512],
                        op0=mybir.AluOpType.add,
                        op1=mybir.AluOpType.mult,
                    )
            wkT = load_wT(wk_d)
            for b in range(GB):
                for tch in range(4):
                    ps = pp.tile([128, 512], f32, tag="pp", name="pp")
                    for d in range(DB):
                        nc.tensor.matmul(
                            ps[:],
                            wkT[d][:, b * 128:(b + 1) * 128],
                            xt[d][:, tch * 512:(tch + 1) * 512],
                            start=(d == 0),
                            stop=(d == DB - 1),
                        )
                    nc.vector.tensor_scalar_add(
                        k_t[b][:, tch * 512:(tch + 1) * 512], ps[:],
                        bk_c[:, b:b + 1]
                    )
            def emit_v_proj():
                # V token-major [t, dout] with a ones column per head
                wvT = load_wT(wv_d)
                for i in range(TB):
                    nc.gpsimd.memset(v_t[i][:], 1.0)
                    ps = pp.tile([128, 512], f32, tag="pp", name="pp")
                    for d in range(DB):
                        nc.tensor.matmul(
                            ps[:],
                            xt[d][:, i * 128:(i + 1) * 128],
                            wvT[d][:],
                            start=(d == 0),
                            stop=(d == DB - 1),
                        )
                    for h in range(HPC):
                        nc.vector.tensor_tensor(
                            v_t[i][:, h * 65:h * 65 + 64],
                            ps[:, h * 64:(h + 1) * 64],
                            bvb[:, h * 64:(h + 1) * 64],
                            op=mybir.AluOpType.add,
                        )

            if not overlap:
                emit_v_proj()
                p1.close()
                maskp.close()
                att_pools = open_att_pools()

        # ---- phase 2: attention; ctx runs 1-2 units behind S/exp ----
        units = [(h, tqh) for h in range(HPC) for tqh in range(2)]
        if True:
            slabp, zp, csp, spp = att_pools
            slabs = {}

            def emit_s_exp(u):
                h, tqh = u
                qk, hb = h // 2, (h % 2) * 64
                t0 = tqh * 1024
                slab = slabp.tile([128, 16 * 1024], bf16, tag="slab",
                                  name="slab")
                slabs[u] = slab
                for j in range(TB):
                    sps = spp.tile([128, 1024], f32, tag="sp", name="sp")
                    for q in range(2):
                        nc.tensor.matmul(
                            sps[:, q * 512:(q + 1) * 512],
                            k_t[qk][hb:hb + 64, j * 128:(j + 1) * 128],
                            q_t[qk][hb:hb + 64,
                                    t0 + q * 512:t0 + (q + 1) * 512],
                            start=True,
                            stop=True,
                        )
                    nc.scalar.activation(
                        slab[:, j * 1024:(j + 1) * 1024],
                        sps[:],
                        mybir.ActivationFunctionType.Exp,
                        scale=SCALE,
                    )

            if overlap:
                # V projection runs under the first two units' exp
                emit_s_exp(units[0])
                emit_s_exp(units[1])
                emit_v_proj()
                p1.close()

            late = ExitStack()
            cpp = late.enter_context(
                tc.tile_pool(name="cpsum", bufs=2, space="PSUM"))
            woTp = late.enter_context(tc.tile_pool(name="wot", bufs=8))
            cf0p = late.enter_context(tc.tile_pool(name="cf0", bufs=1))
            woT = [woTp.tile([128, G], bf16, tag="wot", name="wot")
                   for _ in range(DB)]
            cf0 = [cf0p.tile([128, T], bf16, tag=f"cf0{j}", name=f"cf0{j}")
                   for j in range(GB)]

            def emit_collective(c, cf_tiles):
                if single_core:
                    nc.sync.dma_start(
                        cc_out[c][0:256, :], cc_in[c * 256:(c + 1) * 256, :]
                    )
                    nc.sync.dma_start(
                        cc_out[c][256:512, :], cc_in[c * 256:(c + 1) * 256, :]
                    )
                else:
                    nc.gpsimd.collective_compute(
                        "AllGather",
                        mybir.AluOpType.bypass,
                        replica_groups=[[0, 1], [2, 3], [4, 5], [6, 7]],
                        ins=[cc_in[c * 256:(c + 1) * 256, :].opt()],
                        outs=[cc_out[c][:].opt()],
                    )
                for j in range(GB):
                    nc.sync.dma_start(
                        cf_tiles[j][:], cc_out[c][j * 128:(j + 1) * 128, :]
                    )

            def emit_woT_prep():
                for r in range(GB):
                    ws = csp.tile([128, D], f32, tag="cst", name="wos")
                    nc.sync.dma_start(ws[:], wo_d[r * 128:(r + 1) * 128, :])
                    for d in range(DB):
                        ps = cpp.tile([128, 512], f32, tag="cp", name="cp")
                        nc.tensor.transpose(
                            ps[:, 0:128], ws[:, d * 128:(d + 1) * 128],
                            identity[:]
                        )
                        nc.vector.tensor_copy(
                            woT[d][:, r * 128:(r + 1) * 128], ps[:, 0:128]
                        )

            def emit_ctx(u):
                h, tqh = u
                t0 = tqh * 1024
                slab = slabs.pop(u)
                cps = cpp.tile([65, 1024], f32, tag="cp", name="cp")
                for q in range(2):
                    for j in range(TB):
                        nc.tensor.matmul(
                            cps[:, q * 512:(q + 1) * 512],
                            v_t[j][:, h * 65:h * 65 + 65],
                            slab[:, j * 1024 + q * 512:
                                  j * 1024 + (q + 1) * 512],
                            start=(j == 0),
                            stop=(j == TB - 1),
                        )
                # row 64 holds Z = sum_k exp; scale rows 0..63 by 1/Z
                zrow = zp.tile([128, 1024], f32, tag="z", name="z")
                nc.vector.tensor_copy(zrow[64:65, :], cps[64:65, :])
                nc.sync.dma_start(zrow[0:1, :], zrow[64:65, :])
                nc.vector.reciprocal(zrow[0:1, :], zrow[0:1, :])
                bct = zp.tile([64, 1024], f32, tag="bc", name="bc", bufs=1)
                nc.gpsimd.partition_broadcast(bct[:], zrow[0:1, :])
                cst = csp.tile([64, 1024], bf16, tag="cst", name="cst")
                nc.vector.tensor_tensor(
                    cst[:], cps[0:64, :], bct[:], op=mybir.AluOpType.mult
                )
                nc.sync.dma_start(
                    cc_in[h * 64:(h + 1) * 64, t0:t0 + 1024], cst[:]
                )

            if overlap:
                for idx in range(2, len(units)):
                    emit_ctx(units[idx - 2])
                    emit_s_exp(units[idx])
                    if idx == 6:
                        emit_woT_prep()
                    if idx == 10:
                        emit_collective(0, cf0)
                emit_ctx(units[-2])
                emit_ctx(units[-1])
            else:
                for idx, u in enumerate(units):
                    emit_s_exp(u)
                    if idx > 0:
                        emit_ctx(units[idx - 1])
                    if idx == 4:
                        emit_woT_prep()
                    if idx == 9:
                        emit_collective(0, cf0)
                emit_ctx(units[-1])

            # ---- phase 3: second gather half + output projection ----
            # cf1 reuses a slab slot (attention has drained by now)
            if True:
                cf1_all = slabp.tile([128, GB * T], bf16, tag="slab",
                                     name="cf1")
                cf1 = [cf1_all[:, j * T:(j + 1) * T] for j in range(GB)]
                emit_collective(1, cf1)
                cf = [cf0, cf1]
                for i in range(TB):
                    ps = cpp.tile([128, 512], f32, tag="cp", name="cp")
                    first = True
                    for c in range(2):
                        for j in range(GB):
                            nc.tensor.matmul(
                                ps[:],
                                cf[c][j][:, i * 128:(i + 1) * 128],
                                woT[CC_PERM[c][j]][:],
                                start=first,
                                stop=(c == 1 and j == GB - 1),
                            )
                            first = False
                    os_ = csp.tile([128, G], f32, tag="ostage", name="ostage")
                    nc.vector.tensor_tensor(os_[:], ps[:], bob[:],
                                            op=mybir.AluOpType.add)
                    nc.sync.dma_start(out_d[i * 128:(i + 1) * 128, :], os_[:])
            late.close()


    nc.compile()
    return nc


def shard_inputs(query, mask, Wq, bq, Wk, bk, Wv, bv, Wo, bo):
    in_maps = []
    for c in range(N_CORES):
        n, g = c // 2, c % 2
        sl = slice(g * G, (g + 1) * G)
        in_maps.append(
            {
                "query": np.ascontiguousarray(query[n], dtype=np.float32),
                "mask": np.ascontiguousarray(mask[n], dtype=np.int32),
                "Wq": np.ascontiguousarray(Wq[sl], dtype=np.float32),
                "Wk": np.ascontiguousarray(Wk[sl], dtype=np.float32),
                "Wv": np.ascontiguousarray(Wv[sl], dtype=np.float32),
                "Wo": np.ascontiguousarray(Wo[sl], dtype=np.float32),
                "bq": np.ascontiguousarray(bq[sl], dtype=np.float32),
                "bk": np.ascontiguousarray(bk[sl], dtype=np.float32),
                "bv": np.ascontiguousarray(bv[sl], dtype=np.float32),
                "bo": np.ascontiguousarray(bo[sl], dtype=np.float32),
            }
        )
    return in_maps


def gather_outputs(results):
    out = np.empty((N, T, D), np.float32)
    for c in range(N_CORES):
        n, g = c // 2, c % 2
        out[n][:, g * G:(g + 1) * G] = results[c]["out"]
    return out


def kernel(query, mask, Wq, bq, Wk, bk, Wv, bv, Wo, bo):
    in_maps = shard_inputs(query, mask, Wq, bq, Wk, bk, Wv, bv, Wo, bo)
    nc = build_nc()
    res = run_bass_kernel_spmd(nc, in_maps, list(range(N_CORES)))
    return gather_outputs(res.results)

